# revision 10
# baseline (speedup 1.0000x reference)
"""Bipartite GCN message-passing kernel for 8 Trainium2 NeuronCores.

Math (reference): rst = deg_in^-1/2 * segsum_dst( (node_f @ W_side) * deg_out^-1/2 [src] )
Refactor (projection is linear, graph strictly bipartite):
    rst[d] = ( sum_{e->d} c_e * f_raw[src_e] ) @ W_side(d),
    c_e = deg_out[src]^-1/2 * deg_in[dst]^-1/2  (folded into scatter values on host)

Device pipeline per core (dst slots dealt round-robin by degree rank -> SPMD):
  1. per (window=512 dst slots, table half): dma_gather of bf16 feature rows
     (256B) by src, one gather call per <=16-block segment, spread over the
     4 SWDGE queues (greedy size balance) so descriptor generation runs on
     all 4 Q7 core pairs concurrently (~4x emission throughput; the per-call
     descriptor count must stay well under the per-queue ring capacity).
  2. scatter-matmul: agg_PSUM[128f, 512slot] += M_chunk[128e,128f].T @ S_chunk
     [128e, span] with host-built bf16 S carrying c_e (whole S table resident
     in SBUF, loaded upfront); chunks may straddle slots; spans gap-extended
     so every PSUM column is written.
  3. per-window projection with the side weight (bf16) and fp32 staging out.
Host casts features/weights to bf16, builds idx/S streams, unpermutes output.
"""
import sys
import os

for _p in ("/opt/trn_rl_repo",):
    if _p not in sys.path and os.path.isdir(_p):
        sys.path.insert(0, _p)

import numpy as np

N_U = 50000
N_V = 50000
N = N_U + N_V
D = 128
E = 1600000
N_CORES = 8
HALF = 25000          # int16-safe table window
WIN = 512             # dst slots per PSUM window/bank
P = 128
SEGB = 16             # max gather blocks (128 idx each) per call = 2048 idx
NSLAB = 12            # gather slab ring depth
NQ = 4                # SWDGE queues
SPC = N_U // N_CORES  # 6250 slots per core per phase
NWIN = (SPC + WIN - 1) // WIN   # 13 windows per phase


# ----------------------------------------------------------------- host layout
def _build_layout(src, dst, cout, cin):
    """Canonical (SPMD-identical) schedule + per-core idx/S/output data.

    Returns (sched, per_core).
      sched: {"winpass": [...], "calls": [...], "tot_idx", "tot_scols",
              "smax", "windows": [...]}
      per_core[k]: {"idx": [128, tot_idx//16] i16, "s": [128, tot_scols] f32,
                    "dsts": [phase0 dst map, phase1 dst map]}
    """
    winpass = []           # per (phase, win, pass): dict
    calls = []             # per call: dict
    windows = []           # per global window: {"ns", "slot0", "phase"}
    per_core_idx = [np.zeros(0, np.int16)] * 0

    tot_idx = 0
    tot_scols = 0
    core_idx_parts = [[] for _ in range(N_CORES)]
    core_s_cols = 0
    e_data = []            # per phase: (e_core, e_gpos, e_row, e_scol, e_val, s_half_local)
    per_core_dsts = [[] for _ in range(N_CORES)]

    for phase in range(2):
        if phase == 0:       # dsts are v-nodes, sources u-side
            mask = dst >= N_U
            d_local = dst[mask] - N_U
            s_local = src[mask]
            dst_base = N_U
        else:                # dsts are u-nodes, sources v-side
            mask = dst < N_U
            d_local = dst[mask]
            s_local = src[mask] - N_U
            dst_base = 0
        half = (s_local >= HALF).astype(np.int64)
        s_half_local = (s_local - half * HALF).astype(np.int16)

        a_cnt = np.bincount(d_local[half == 0], minlength=N_U)
        b_cnt = np.bincount(d_local[half == 1], minlength=N_U)

        order = np.lexsort((np.arange(N_U), b_cnt, a_cnt))
        rank = np.empty(N_U, np.int64)
        rank[order] = np.arange(N_U)

        # canonical per-slot degrees = max over cores
        a_mat = np.zeros((N_CORES, SPC), np.int64)
        b_mat = np.zeros((N_CORES, SPC), np.int64)
        dst_mat = np.empty((N_CORES, SPC), np.int64)
        r = np.arange(N_U)
        a_mat[r % N_CORES, r // N_CORES] = a_cnt[order]
        b_mat[r % N_CORES, r // N_CORES] = b_cnt[order]
        dst_mat[r % N_CORES, r // N_CORES] = order + dst_base
        A = a_mat.max(axis=0)
        B = b_mat.max(axis=0)
        for k in range(N_CORES):
            per_core_dsts[k].append(dst_mat[k])

        # canonical chunking / spans / segments per (window, pass)
        pos_base = [np.zeros(SPC, np.int64), np.zeros(SPC, np.int64)]
        wp_meta = [[None, None] for _ in range(NWIN)]
        for w in range(NWIN):
            s0, s1 = w * WIN, min((w + 1) * WIN, SPC)
            nsl = s1 - s0
            if phase == 0 and len(windows) <= w + 0:
                pass
            for p_i, C in enumerate((A, B)):
                Cw = C[s0:s1]
                n = int(Cw.sum())
                assert n > 0
                cum = np.cumsum(Cw)
                pos_base[p_i][s0:s1] = np.r_[0, cum[:-1]]
                nb = (n + P - 1) // P
                # slot (window-local) of each canonical position
                slot_of = np.repeat(np.arange(nsl), Cw)
                chunks = []
                prev_end = -1
                sc = 0
                for kblk in range(nb):
                    lo = kblk * P
                    hi = min((kblk + 1) * P, n) - 1
                    st = min(int(slot_of[lo]), prev_end + 1)
                    en = int(slot_of[hi]) if kblk < nb - 1 else nsl - 1
                    en = max(en, st)
                    chunks.append({"st": st, "en": en, "sc": sc})
                    sc += en - st + 1
                    prev_end = en
                swidth = sc
                nseg = (nb + SEGB - 1) // SEGB
                base_sz = nb // nseg
                extra = nb - base_sz * nseg
                segs = []
                b0 = 0
                for si in range(nseg):
                    nbk = base_sz + (1 if si < extra else 0)
                    segs.append((b0, nbk))
                    b0 += nbk
                wp_meta[w][p_i] = {
                    "n": n, "nb": nb, "chunks": chunks, "swidth": swidth,
                    "segs": segs,
                    "idx_off": tot_idx, "scol_off": tot_scols,
                    "phase": phase, "w": w, "p": p_i, "nsl": nsl,
                }
                winpass.append(wp_meta[w][p_i])
                for b0, nbk in segs:
                    calls.append({
                        "phase": phase, "w": w, "p": p_i,
                        "blk0": b0, "nblk": nbk,
                        "icol": (tot_idx + b0 * P) // 16,
                        "n": nbk * P,
                        "wp": len(winpass) - 1,
                    })
                tot_idx += nb * P
                tot_scols += swidth
            windows.append({
                "ns": s1 - s0, "slot0": phase * SPC + s0, "phase": phase,
            })

        # ---- per-core edge placement (vectorized)
        grp = d_local * 2 + half
        sort_i = np.argsort(grp, kind="stable")
        grp_s = grp[sort_i]
        starts = np.r_[0, np.nonzero(np.diff(grp_s))[0] + 1]
        group_id = np.cumsum(np.r_[0, (np.diff(grp_s) != 0).astype(np.int64)])
        within = np.arange(len(grp_s)) - starts[group_id]
        e_rank = np.empty(len(grp), np.int64)
        e_rank[sort_i] = within

        e_rankd = rank[d_local]
        e_core = e_rankd % N_CORES
        e_slot = e_rankd // N_CORES
        e_win = e_slot // WIN
        e_sl_in_win = e_slot - e_win * WIN

        # position within the (win, pass) stream
        pb = np.where(half == 0, pos_base[0][e_slot], pos_base[1][e_slot])
        e_pos = pb + e_rank

        idx_off_map = np.zeros((NWIN, 2), np.int64)
        scol_off_map = np.zeros((NWIN, 2), np.int64)
        for w in range(NWIN):
            for p_i in (0, 1):
                idx_off_map[w, p_i] = wp_meta[w][p_i]["idx_off"]
                scol_off_map[w, p_i] = wp_meta[w][p_i]["scol_off"]

        e_gpos = idx_off_map[e_win, half] + e_pos

        # chunk lookup for scol: chunk = e_pos // P within (win, pass)
        # need chunk span starts: build flat arrays per (win, pass)
        ch_st_flat = {}
        for w in range(NWIN):
            for p_i in (0, 1):
                m = wp_meta[w][p_i]
                ch_st_flat[(w, p_i)] = (
                    np.array([c["st"] for c in m["chunks"]], np.int64),
                    np.array([c["sc"] for c in m["chunks"]], np.int64),
                )
        e_chunk = e_pos // P
        e_scol = np.empty(len(grp), np.int64)
        for w in range(NWIN):
            for p_i in (0, 1):
                m2 = (e_win == w) & (half == p_i)
                if not m2.any():
                    continue
                st_arr, sc_arr = ch_st_flat[(w, p_i)]
                ch = e_chunk[m2]
                e_scol[m2] = (scol_off_map[w, p_i] + sc_arr[ch]
                              + e_sl_in_win[m2] - st_arr[ch])

        e_val = (cout[s_local + (0 if phase == 0 else N_U)]
                 * cin[d_local + dst_base]).astype(np.float32)
        e_data.append((e_core, e_gpos, e_pos % P, e_scol, e_val, s_half_local))

    # ---- build per-core flat arrays
    per_core = []
    for k in range(N_CORES):
        idx_flat = np.zeros(tot_idx, np.int16)
        sval = np.zeros((P, tot_scols), np.float32)
        for (e_core, e_gpos, e_row, e_scol, e_val, shl) in e_data:
            m = e_core == k
            idx_flat[e_gpos[m]] = shl[m]
            sval[e_row[m], e_scol[m]] = e_val[m]
        # wrap idx per call into [16, n/16] tiled x8
        cols = []
        for c in calls:
            wpm = winpass[c["wp"]]
            a = wpm["idx_off"] + c["blk0"] * P
            seg = idx_flat[a:a + c["n"]]
            t = seg.reshape(c["n"] // 16, 16).T
            cols.append(np.tile(t, (N_CORES, 1)))
        idx_arr = np.ascontiguousarray(np.concatenate(cols, axis=1))
        per_core.append({"idx": idx_arr, "s": sval,
                         "dsts": per_core_dsts[k]})

    qload = [0] * NQ
    for c in calls:
        q = min(range(NQ), key=lambda i: qload[i])
        qload[q] += c["n"]
        c["q"] = q
    smax = max(m["swidth"] for m in winpass)
    nbmax = max(min(SEGB, m["nb"]) for m in winpass)
    sched = {"winpass": winpass, "calls": calls, "windows": windows,
             "tot_idx": tot_idx, "tot_scols": tot_scols, "smax": smax,
             "nbmax": nbmax}
    return sched, per_core


# ------------------------------------------------------------------ device code
def _build_nc(sched):
    import concourse.bacc as bacc
    import concourse.bass as bass
    import concourse.mybir as mybir
    from concourse._compat import get_trn_type
    from concourse.library_config import mlp

    nc = bacc.Bacc(get_trn_type() or "TRN2", target_bir_lowering=False,
                   debug=False, num_swdge_queues=NQ)
    f32 = mybir.dt.float32
    bf16 = mybir.dt.bfloat16
    i16 = mybir.dt.int16

    u16 = nc.dram_tensor("u16", [N_U, D], bf16, kind="ExternalInput")
    v16 = nc.dram_tensor("v16", [N_V, D], bf16, kind="ExternalInput")
    uw = nc.dram_tensor("uw", [D, D], bf16, kind="ExternalInput")
    vw = nc.dram_tensor("vw", [D, D], bf16, kind="ExternalInput")

    calls = sched["calls"]
    winpass = sched["winpass"]
    windows = sched["windows"]
    tot_idx = sched["tot_idx"]
    tot_scols = sched["tot_scols"]
    smax = sched["smax"]
    NW = len(windows)
    NC_ = len(calls)
    NWP = len(winpass)

    idx_in = nc.dram_tensor("idx", [P, tot_idx // 16], i16, kind="ExternalInput")
    s_in = nc.dram_tensor("sval", [P, tot_scols], bf16, kind="ExternalInput")
    out = nc.dram_tensor("out", [P, 2 * SPC], f32, kind="ExternalOutput")

    idx_sb = nc.alloc_sbuf_tensor("idx_sb", [P, tot_idx // 16], i16)
    slabs = [nc.alloc_sbuf_tensor(f"m{i}", [P, SEGB, P], bf16)
             for i in range(NSLAB)]
    s_all = nc.alloc_sbuf_tensor("s_all", [P, tot_scols], bf16)
    agg_sb = [nc.alloc_sbuf_tensor(f"agg{i}", [P, WIN], bf16) for i in (0, 1)]
    stage = [nc.alloc_sbuf_tensor(f"st{i}", [P, WIN], f32) for i in (0, 1)]
    w_sb = [nc.alloc_sbuf_tensor(f"w{i}", [P, D], bf16) for i in (0, 1)]

    agg_ps = [nc.alloc_psum_tensor(f"aps{i}", [P, WIN], f32) for i in (0, 1)]
    proj_ps = [nc.alloc_psum_tensor(f"pps{i}", [P, WIN], f32) for i in (0, 1)]

    sem_idx = nc.alloc_semaphore("idxld")
    sem_ld = nc.alloc_semaphore("wld")
    sem_q = [nc.alloc_semaphore(f"q{i}") for i in range(NQ)]
    sem_s = nc.alloc_semaphore("ssem")
    sem_mm = nc.alloc_semaphore("mmcall")     # +1 per consumed call (tensor)
    sem_mmw = nc.alloc_semaphore("mmwin")     # +1 per window agg done
    sem_agg = nc.alloc_semaphore("aggsem")
    sem_proj = nc.alloc_semaphore("projsem")
    sem_stage = nc.alloc_semaphore("stsem")
    sem_out = nc.alloc_semaphore("outsem")

    # queue completion targets per call
    q_target = [0] * NC_
    q_cnt = [0] * NQ
    for c_i in range(NC_):
        q = calls[c_i]["q"]
        q_cnt[q] += 16
        q_target[c_i] = q_cnt[q]
    # map call -> window index (global), and window -> first/last call
    def win_g(c):
        return c["phase"] * NWIN + c["w"]
    win_first_call = {}
    win_last_call = {}
    for c_i, c in enumerate(calls):
        wg = win_g(c)
        if wg not in win_first_call:
            win_first_call[wg] = c_i
        win_last_call[wg] = c_i

    idx_split = calls[min(6, NC_ - 1)]["icol"]
    if idx_split == 0:
        idx_split = tot_idx // 16

    with nc.Block() as block:
        @block.sync
        def _(sy: bass.BassEngine):
            sy.dma_start(idx_sb[:, :idx_split],
                         idx_in[:, :idx_split]).then_inc(sem_idx, 16)
            sy.dma_start(idx_sb[:, idx_split:],
                         idx_in[:, idx_split:]).then_inc(sem_idx, 16)
            sy.dma_start(w_sb[0][:], uw[:]).then_inc(sem_ld, 16)
            sy.dma_start(w_sb[1][:], vw[:]).then_inc(sem_ld, 16)
            s_split = winpass[min(8, NWP - 1)]["scol_off"] or tot_scols
            sy.dma_start(s_all[:, :s_split],
                         s_in[:, :s_split]).then_inc(sem_s, 16)
            sy.dma_start(s_all[:, s_split:],
                         s_in[:, s_split:]).then_inc(sem_s, 16)
            sy.wait_ge(sem_out, NW * 16)

        @block.gpsimd
        def _(gp: bass.BassGpSimd):
            gp.load_library(mlp)
            gp.wait_ge(sem_idx, 16)
            for c_i, c in enumerate(calls):
                if c_i == 6:
                    gp.wait_ge(sem_idx, 32)
                if c_i >= NSLAB:
                    gp.wait_ge(sem_mm, c_i - NSLAB + 1)
                m = winpass[c["wp"]]
                if c["phase"] == 0:
                    tab = u16[0:HALF, :] if c["p"] == 0 else u16[HALF:N_U, :]
                else:
                    tab = v16[0:HALF, :] if c["p"] == 0 else v16[HALF:N_V, :]
                gp.dma_gather(
                    slabs[c_i % NSLAB][:, :c["nblk"], :],
                    tab,
                    idx_sb[:, c["icol"]:c["icol"] + c["n"] // 16],
                    c["n"], c["n"], D,
                    single_packet=False,
                    queue_num=c["q"],
                ).then_inc(sem_q[c["q"]], 16)

        @block.tensor
        def _(te):
            te.wait_ge(sem_ld, 32)
            s_seen = {}
            for c_i, c in enumerate(calls):
                m = winpass[c["wp"]]
                wg = win_g(c)
                te.wait_ge(sem_q[c["q"]], q_target[c_i])
                if c["wp"] not in s_seen:
                    te.wait_ge(sem_s, 16 if c["wp"] < 8 else 32)
                    s_seen[c["wp"]] = True
                if c_i == win_first_call[wg] and wg >= 2:
                    te.wait_ge(sem_agg, wg - 1)
                b = wg % 2
                first_of_win = c_i == win_first_call[wg] and c["p"] == 0 \
                    and c["blk0"] == 0
                for kb in range(c["nblk"]):
                    blk = c["blk0"] + kb
                    ch = m["chunks"][blk]
                    span = ch["en"] - ch["st"] + 1
                    is_first = first_of_win and kb == 0
                    is_last = (c_i == win_last_call[wg]
                               and kb == c["nblk"] - 1)
                    mm = te.matmul(
                        out=agg_ps[b][:, ch["st"]:ch["en"] + 1],
                        lhsT=slabs[c_i % NSLAB][:, kb, :],
                        rhs=s_all[:, m["scol_off"] + ch["sc"]:
                                  m["scol_off"] + ch["sc"] + span],
                        start=is_first,
                        stop=is_last,
                    )
                    if is_last:
                        mm.then_inc(sem_mmw, 1)
                te.sem_inc(sem_mm, 1)
                if c_i == win_last_call[wg]:
                    # projection for window wg
                    te.wait_ge(sem_agg, wg + 1)
                    if wg >= 2:
                        te.wait_ge(sem_stage, wg - 1)
                    te.matmul(
                        out=proj_ps[b][:, :windows[wg]["ns"]],
                        lhsT=w_sb[windows[wg]["phase"]][:],
                        rhs=agg_sb[b][:, :windows[wg]["ns"]],
                        start=True, stop=True,
                    ).then_inc(sem_proj, 1)

        @block.vector
        def _(ve):
            for wg in range(NW):
                b = wg % 2
                ns = windows[wg]["ns"]
                ve.wait_ge(sem_mmw, wg + 1)
                ve.tensor_copy(out=agg_sb[b][:, :ns],
                               in_=agg_ps[b][:, :ns]).then_inc(sem_agg, 1)
                ve.wait_ge(sem_proj, wg + 1)
                if wg >= 2:
                    ve.wait_ge(sem_out, 16 * (wg - 1))
                ve.tensor_copy(out=stage[b][:, :ns],
                               in_=proj_ps[b][:, :ns]).then_inc(sem_stage, 1)

        @block.scalar
        def _(sc):
            for wg in range(NW):
                b = wg % 2
                ns = windows[wg]["ns"]
                s0 = windows[wg]["slot0"]
                sc.wait_ge(sem_stage, wg + 1)
                sc.dma_start(out[:, s0:s0 + ns],
                             stage[b][:, :ns]).then_inc(sem_out, 16)

    nc.compile()
    return nc


# ---------------------------------------------------------------------- kernel
def kernel(u_f, v_f, u_w, v_w, src, dst):
    import ml_dtypes
    from concourse.bass_utils import run_bass_kernel_spmd

    src = np.asarray(src)
    dst = np.asarray(dst)
    u_f = np.asarray(u_f, np.float32)
    v_f = np.asarray(v_f, np.float32)

    deg_out = np.bincount(src, minlength=N).astype(np.float32)
    deg_in = np.bincount(dst, minlength=N).astype(np.float32)
    cout = np.maximum(deg_out, 1.0) ** -0.5
    cin = np.maximum(deg_in, 1.0) ** -0.5

    sched, per_core = _build_layout(src, dst, cout, cin)

    nc = _build_nc(sched)
    bf = ml_dtypes.bfloat16
    u16 = u_f.astype(bf)
    v16 = v_f.astype(bf)
    uw16 = np.asarray(u_w, np.float32).astype(bf)
    vw16 = np.asarray(v_w, np.float32).astype(bf)
    in_maps = []
    for k in range(N_CORES):
        in_maps.append({
            "u16": u16, "v16": v16, "uw": uw16, "vw": vw16,
            "idx": per_core[k]["idx"],
            "sval": per_core[k]["s"].astype(bf),
        })
    trace = bool(os.environ.get("KERNEL_TRACE"))
    res = run_bass_kernel_spmd(nc, in_maps, core_ids=list(range(N_CORES)),
                               trace=trace)
    if trace:
        print(f"HW exec time: {res.exec_time_ns} ns")
        kernel.last_profile = res.profile_json

    out_full = np.zeros((N, D), np.float32)
    for k in range(N_CORES):
        fm = res.results[k]["out"]            # [128, 2*SPC] feat-major
        rows = np.ascontiguousarray(fm.T)     # [2*SPC, 128]
        for phase in range(2):
            dsts = per_core[k]["dsts"][phase]
            out_full[dsts] = rows[phase * SPC:(phase + 1) * SPC]
    return out_full


# revision 11
# speedup vs baseline: 1.0065x; 1.0065x over previous
"""Bipartite GCN message-passing kernel for 8 Trainium2 NeuronCores.

Math (reference): rst = deg_in^-1/2 * segsum_dst( (node_f @ W_side) * deg_out^-1/2 [src] )
Refactor (projection is linear, graph strictly bipartite):
    rst[d] = ( sum_{e->d} c_e * f_raw[src_e] ) @ W_side(d),
    c_e = deg_out[src]^-1/2 * deg_in[dst]^-1/2  (folded into scatter values on host)

Device pipeline per core (dst slots dealt round-robin by degree rank -> SPMD):
  1. per (window=512 dst slots, table half): dma_gather of bf16 feature rows
     (256B) by src, one gather call per <=16-block segment, spread over the
     4 SWDGE queues (greedy size balance) so descriptor generation runs on
     all 4 Q7 core pairs concurrently (~4x emission throughput; the per-call
     descriptor count must stay well under the per-queue ring capacity).
  2. scatter-matmul: agg_PSUM[128f, 512slot] += M_chunk[128e,128f].T @ S_chunk
     [128e, span] with host-built bf16 S carrying c_e (whole S table resident
     in SBUF, loaded upfront); chunks may straddle slots; spans gap-extended
     so every PSUM column is written.
  3. per-window projection with the side weight (bf16) and fp32 staging out.
Host casts features/weights to bf16, builds idx/S streams, unpermutes output.
"""
import sys
import os

for _p in ("/opt/trn_rl_repo",):
    if _p not in sys.path and os.path.isdir(_p):
        sys.path.insert(0, _p)

import numpy as np

N_U = 50000
N_V = 50000
N = N_U + N_V
D = 128
E = 1600000
N_CORES = 8
HALF = 25000          # int16-safe table window
WIN = 512             # dst slots per PSUM window/bank
P = 128
SEGB = 16             # max gather blocks (128 idx each) per call = 2048 idx
NSLAB = 12            # gather slab ring depth
NQ = 4                # SWDGE queues
SPC = N_U // N_CORES  # 6250 slots per core per phase
NWIN = (SPC + WIN - 1) // WIN   # 13 windows per phase


# ----------------------------------------------------------------- host layout
def _build_layout(src, dst, cout, cin):
    """Canonical (SPMD-identical) schedule + per-core idx/S/output data.

    Returns (sched, per_core).
      sched: {"winpass": [...], "calls": [...], "tot_idx", "tot_scols",
              "smax", "windows": [...]}
      per_core[k]: {"idx": [128, tot_idx//16] i16, "s": [128, tot_scols] f32,
                    "dsts": [phase0 dst map, phase1 dst map]}
    """
    winpass = []           # per (phase, win, pass): dict
    calls = []             # per call: dict
    windows = []           # per global window: {"ns", "slot0", "phase"}
    per_core_idx = [np.zeros(0, np.int16)] * 0

    tot_idx = 0
    tot_scols = 0
    core_idx_parts = [[] for _ in range(N_CORES)]
    core_s_cols = 0
    e_data = []            # per phase: (e_core, e_gpos, e_row, e_scol, e_val, s_half_local)
    per_core_dsts = [[] for _ in range(N_CORES)]

    for phase in range(2):
        if phase == 0:       # dsts are v-nodes, sources u-side
            mask = dst >= N_U
            d_local = dst[mask] - N_U
            s_local = src[mask]
            dst_base = N_U
        else:                # dsts are u-nodes, sources v-side
            mask = dst < N_U
            d_local = dst[mask]
            s_local = src[mask] - N_U
            dst_base = 0
        half = (s_local >= HALF).astype(np.int64)
        s_half_local = (s_local - half * HALF).astype(np.int16)

        a_cnt = np.bincount(d_local[half == 0], minlength=N_U)
        b_cnt = np.bincount(d_local[half == 1], minlength=N_U)

        order = np.lexsort((np.arange(N_U), b_cnt, a_cnt))
        rank = np.empty(N_U, np.int64)
        rank[order] = np.arange(N_U)

        # canonical per-slot degrees = max over cores
        a_mat = np.zeros((N_CORES, SPC), np.int64)
        b_mat = np.zeros((N_CORES, SPC), np.int64)
        dst_mat = np.empty((N_CORES, SPC), np.int64)
        r = np.arange(N_U)
        a_mat[r % N_CORES, r // N_CORES] = a_cnt[order]
        b_mat[r % N_CORES, r // N_CORES] = b_cnt[order]
        dst_mat[r % N_CORES, r // N_CORES] = order + dst_base
        A = a_mat.max(axis=0)
        B = b_mat.max(axis=0)
        for k in range(N_CORES):
            per_core_dsts[k].append(dst_mat[k])

        # canonical chunking / spans / segments per (window, pass)
        pos_base = [np.zeros(SPC, np.int64), np.zeros(SPC, np.int64)]
        wp_meta = [[None, None] for _ in range(NWIN)]
        for w in range(NWIN):
            s0, s1 = w * WIN, min((w + 1) * WIN, SPC)
            nsl = s1 - s0
            if phase == 0 and len(windows) <= w + 0:
                pass
            for p_i, C in enumerate((A, B)):
                Cw = C[s0:s1]
                n = int(Cw.sum())
                assert n > 0
                cum = np.cumsum(Cw)
                pos_base[p_i][s0:s1] = np.r_[0, cum[:-1]]
                nb = (n + P - 1) // P
                # slot (window-local) of each canonical position
                slot_of = np.repeat(np.arange(nsl), Cw)
                chunks = []
                prev_end = -1
                sc = 0
                for kblk in range(nb):
                    lo = kblk * P
                    hi = min((kblk + 1) * P, n) - 1
                    st = min(int(slot_of[lo]), prev_end + 1)
                    en = int(slot_of[hi]) if kblk < nb - 1 else nsl - 1
                    en = max(en, st)
                    chunks.append({"st": st, "en": en, "sc": sc})
                    sc += en - st + 1
                    prev_end = en
                swidth = sc
                last_wp = phase == 1 and w == NWIN - 1 and p_i == 1
                nbm = nb - 1 if (last_wp and nb > 1) else nb
                nseg = (nbm + SEGB - 1) // SEGB
                base_sz = nbm // nseg
                extra = nbm - base_sz * nseg
                segs = []
                b0 = 0
                for si in range(nseg):
                    nbk = base_sz + (1 if si < extra else 0)
                    segs.append((b0, nbk))
                    b0 += nbk
                if last_wp and nb > 1:
                    # tiny final call: shortens the post-stream drain tail
                    segs.append((b0, 1))
                wp_meta[w][p_i] = {
                    "n": n, "nb": nb, "chunks": chunks, "swidth": swidth,
                    "segs": segs,
                    "idx_off": tot_idx, "scol_off": tot_scols,
                    "phase": phase, "w": w, "p": p_i, "nsl": nsl,
                }
                winpass.append(wp_meta[w][p_i])
                for b0, nbk in segs:
                    calls.append({
                        "phase": phase, "w": w, "p": p_i,
                        "blk0": b0, "nblk": nbk,
                        "icol": (tot_idx + b0 * P) // 16,
                        "n": nbk * P,
                        "wp": len(winpass) - 1,
                    })
                tot_idx += nb * P
                tot_scols += swidth
            windows.append({
                "ns": s1 - s0, "slot0": phase * SPC + s0, "phase": phase,
            })

        # ---- per-core edge placement (vectorized)
        grp = d_local * 2 + half
        sort_i = np.argsort(grp, kind="stable")
        grp_s = grp[sort_i]
        starts = np.r_[0, np.nonzero(np.diff(grp_s))[0] + 1]
        group_id = np.cumsum(np.r_[0, (np.diff(grp_s) != 0).astype(np.int64)])
        within = np.arange(len(grp_s)) - starts[group_id]
        e_rank = np.empty(len(grp), np.int64)
        e_rank[sort_i] = within

        e_rankd = rank[d_local]
        e_core = e_rankd % N_CORES
        e_slot = e_rankd // N_CORES
        e_win = e_slot // WIN
        e_sl_in_win = e_slot - e_win * WIN

        # position within the (win, pass) stream
        pb = np.where(half == 0, pos_base[0][e_slot], pos_base[1][e_slot])
        e_pos = pb + e_rank

        idx_off_map = np.zeros((NWIN, 2), np.int64)
        scol_off_map = np.zeros((NWIN, 2), np.int64)
        for w in range(NWIN):
            for p_i in (0, 1):
                idx_off_map[w, p_i] = wp_meta[w][p_i]["idx_off"]
                scol_off_map[w, p_i] = wp_meta[w][p_i]["scol_off"]

        e_gpos = idx_off_map[e_win, half] + e_pos

        # chunk lookup for scol: chunk = e_pos // P within (win, pass)
        # need chunk span starts: build flat arrays per (win, pass)
        ch_st_flat = {}
        for w in range(NWIN):
            for p_i in (0, 1):
                m = wp_meta[w][p_i]
                ch_st_flat[(w, p_i)] = (
                    np.array([c["st"] for c in m["chunks"]], np.int64),
                    np.array([c["sc"] for c in m["chunks"]], np.int64),
                )
        e_chunk = e_pos // P
        e_scol = np.empty(len(grp), np.int64)
        for w in range(NWIN):
            for p_i in (0, 1):
                m2 = (e_win == w) & (half == p_i)
                if not m2.any():
                    continue
                st_arr, sc_arr = ch_st_flat[(w, p_i)]
                ch = e_chunk[m2]
                e_scol[m2] = (scol_off_map[w, p_i] + sc_arr[ch]
                              + e_sl_in_win[m2] - st_arr[ch])

        e_val = (cout[s_local + (0 if phase == 0 else N_U)]
                 * cin[d_local + dst_base]).astype(np.float32)
        e_data.append((e_core, e_gpos, e_pos % P, e_scol, e_val, s_half_local))

    # ---- build per-core flat arrays
    per_core = []
    for k in range(N_CORES):
        idx_flat = np.zeros(tot_idx, np.int16)
        sval = np.zeros((P, tot_scols), np.float32)
        for (e_core, e_gpos, e_row, e_scol, e_val, shl) in e_data:
            m = e_core == k
            idx_flat[e_gpos[m]] = shl[m]
            sval[e_row[m], e_scol[m]] = e_val[m]
        # wrap idx per call into [16, n/16] tiled x8
        cols = []
        for c in calls:
            wpm = winpass[c["wp"]]
            a = wpm["idx_off"] + c["blk0"] * P
            seg = idx_flat[a:a + c["n"]]
            t = seg.reshape(c["n"] // 16, 16).T
            cols.append(np.tile(t, (N_CORES, 1)))
        idx_arr = np.ascontiguousarray(np.concatenate(cols, axis=1))
        per_core.append({"idx": idx_arr, "s": sval,
                         "dsts": per_core_dsts[k]})

    qload = [0] * NQ
    for c in calls:
        q = min(range(NQ), key=lambda i: qload[i])
        qload[q] += c["n"]
        c["q"] = q
    smax = max(m["swidth"] for m in winpass)
    nbmax = max(min(SEGB, m["nb"]) for m in winpass)
    sched = {"winpass": winpass, "calls": calls, "windows": windows,
             "tot_idx": tot_idx, "tot_scols": tot_scols, "smax": smax,
             "nbmax": nbmax}
    return sched, per_core


# ------------------------------------------------------------------ device code
def _build_nc(sched):
    import concourse.bacc as bacc
    import concourse.bass as bass
    import concourse.mybir as mybir
    from concourse._compat import get_trn_type
    from concourse.library_config import mlp

    nc = bacc.Bacc(get_trn_type() or "TRN2", target_bir_lowering=False,
                   debug=False, num_swdge_queues=NQ)
    f32 = mybir.dt.float32
    bf16 = mybir.dt.bfloat16
    i16 = mybir.dt.int16

    u16 = nc.dram_tensor("u16", [N_U, D], bf16, kind="ExternalInput")
    v16 = nc.dram_tensor("v16", [N_V, D], bf16, kind="ExternalInput")
    uw = nc.dram_tensor("uw", [D, D], bf16, kind="ExternalInput")
    vw = nc.dram_tensor("vw", [D, D], bf16, kind="ExternalInput")

    calls = sched["calls"]
    winpass = sched["winpass"]
    windows = sched["windows"]
    tot_idx = sched["tot_idx"]
    tot_scols = sched["tot_scols"]
    smax = sched["smax"]
    NW = len(windows)
    NC_ = len(calls)
    NWP = len(winpass)

    idx_in = nc.dram_tensor("idx", [P, tot_idx // 16], i16, kind="ExternalInput")
    s_in = nc.dram_tensor("sval", [P, tot_scols], bf16, kind="ExternalInput")
    out = nc.dram_tensor("out", [P, 2 * SPC], f32, kind="ExternalOutput")

    idx_sb = nc.alloc_sbuf_tensor("idx_sb", [P, tot_idx // 16], i16)
    slabs = [nc.alloc_sbuf_tensor(f"m{i}", [P, SEGB, P], bf16)
             for i in range(NSLAB)]
    s_all = nc.alloc_sbuf_tensor("s_all", [P, tot_scols], bf16)
    agg_sb = [nc.alloc_sbuf_tensor(f"agg{i}", [P, WIN], bf16) for i in (0, 1)]
    stage = [nc.alloc_sbuf_tensor(f"st{i}", [P, WIN], f32) for i in (0, 1)]
    w_sb = [nc.alloc_sbuf_tensor(f"w{i}", [P, D], bf16) for i in (0, 1)]

    agg_ps = [nc.alloc_psum_tensor(f"aps{i}", [P, WIN], f32) for i in (0, 1)]
    proj_ps = [nc.alloc_psum_tensor(f"pps{i}", [P, WIN], f32) for i in (0, 1)]

    sem_idx = nc.alloc_semaphore("idxld")
    sem_ld = nc.alloc_semaphore("wld")
    sem_q = [nc.alloc_semaphore(f"q{i}") for i in range(NQ)]
    sem_s = nc.alloc_semaphore("ssem")
    sem_mm = nc.alloc_semaphore("mmcall")     # +1 per consumed call (tensor)
    sem_mmw = nc.alloc_semaphore("mmwin")     # +1 per window agg done
    sem_agg = nc.alloc_semaphore("aggsem")
    sem_proj = nc.alloc_semaphore("projsem")
    sem_stage = nc.alloc_semaphore("stsem")
    sem_out = nc.alloc_semaphore("outsem")

    # queue completion targets per call
    q_target = [0] * NC_
    q_cnt = [0] * NQ
    for c_i in range(NC_):
        q = calls[c_i]["q"]
        q_cnt[q] += 16
        q_target[c_i] = q_cnt[q]
    # map call -> window index (global), and window -> first/last call
    def win_g(c):
        return c["phase"] * NWIN + c["w"]
    win_first_call = {}
    win_last_call = {}
    for c_i, c in enumerate(calls):
        wg = win_g(c)
        if wg not in win_first_call:
            win_first_call[wg] = c_i
        win_last_call[wg] = c_i

    idx_split = calls[min(6, NC_ - 1)]["icol"]
    if idx_split == 0:
        idx_split = tot_idx // 16

    with nc.Block() as block:
        @block.sync
        def _(sy: bass.BassEngine):
            sy.dma_start(idx_sb[:, :idx_split],
                         idx_in[:, :idx_split]).then_inc(sem_idx, 16)
            sy.dma_start(idx_sb[:, idx_split:],
                         idx_in[:, idx_split:]).then_inc(sem_idx, 16)
            sy.dma_start(w_sb[0][:], uw[:]).then_inc(sem_ld, 16)
            sy.dma_start(w_sb[1][:], vw[:]).then_inc(sem_ld, 16)
            s_split = winpass[min(8, NWP - 1)]["scol_off"] or tot_scols
            sy.dma_start(s_all[:, :s_split],
                         s_in[:, :s_split]).then_inc(sem_s, 16)
            sy.dma_start(s_all[:, s_split:],
                         s_in[:, s_split:]).then_inc(sem_s, 16)
            sy.wait_ge(sem_out, NW * 16)

        @block.gpsimd
        def _(gp: bass.BassGpSimd):
            gp.load_library(mlp)
            gp.wait_ge(sem_idx, 16)
            for c_i, c in enumerate(calls):
                if c_i == 6:
                    gp.wait_ge(sem_idx, 32)
                if c_i >= NSLAB:
                    gp.wait_ge(sem_mm, c_i - NSLAB + 1)
                m = winpass[c["wp"]]
                if c["phase"] == 0:
                    tab = u16[0:HALF, :] if c["p"] == 0 else u16[HALF:N_U, :]
                else:
                    tab = v16[0:HALF, :] if c["p"] == 0 else v16[HALF:N_V, :]
                gp.dma_gather(
                    slabs[c_i % NSLAB][:, :c["nblk"], :],
                    tab,
                    idx_sb[:, c["icol"]:c["icol"] + c["n"] // 16],
                    c["n"], c["n"], D,
                    single_packet=False,
                    queue_num=c["q"],
                ).then_inc(sem_q[c["q"]], 16)

        @block.tensor
        def _(te):
            te.wait_ge(sem_ld, 32)
            s_seen = {}
            for c_i, c in enumerate(calls):
                m = winpass[c["wp"]]
                wg = win_g(c)
                te.wait_ge(sem_q[c["q"]], q_target[c_i])
                if c["wp"] not in s_seen:
                    te.wait_ge(sem_s, 16 if c["wp"] < 8 else 32)
                    s_seen[c["wp"]] = True
                if c_i == win_first_call[wg] and wg >= 2:
                    te.wait_ge(sem_agg, wg - 1)
                b = wg % 2
                first_of_win = c_i == win_first_call[wg] and c["p"] == 0 \
                    and c["blk0"] == 0
                for kb in range(c["nblk"]):
                    blk = c["blk0"] + kb
                    ch = m["chunks"][blk]
                    span = ch["en"] - ch["st"] + 1
                    is_first = first_of_win and kb == 0
                    is_last = (c_i == win_last_call[wg]
                               and kb == c["nblk"] - 1)
                    mm = te.matmul(
                        out=agg_ps[b][:, ch["st"]:ch["en"] + 1],
                        lhsT=slabs[c_i % NSLAB][:, kb, :],
                        rhs=s_all[:, m["scol_off"] + ch["sc"]:
                                  m["scol_off"] + ch["sc"] + span],
                        start=is_first,
                        stop=is_last,
                    )
                    if is_last:
                        mm.then_inc(sem_mmw, 1)
                te.sem_inc(sem_mm, 1)
                if c_i == win_last_call[wg]:
                    # projection for window wg
                    te.wait_ge(sem_agg, wg + 1)
                    if wg >= 2:
                        te.wait_ge(sem_stage, wg - 1)
                    te.matmul(
                        out=proj_ps[b][:, :windows[wg]["ns"]],
                        lhsT=w_sb[windows[wg]["phase"]][:],
                        rhs=agg_sb[b][:, :windows[wg]["ns"]],
                        start=True, stop=True,
                    ).then_inc(sem_proj, 1)

        @block.vector
        def _(ve):
            for wg in range(NW):
                b = wg % 2
                ns = windows[wg]["ns"]
                ve.wait_ge(sem_mmw, wg + 1)
                ve.tensor_copy(out=agg_sb[b][:, :ns],
                               in_=agg_ps[b][:, :ns]).then_inc(sem_agg, 1)
                ve.wait_ge(sem_proj, wg + 1)
                if wg >= 2:
                    ve.wait_ge(sem_out, 16 * (wg - 1))
                ve.tensor_copy(out=stage[b][:, :ns],
                               in_=proj_ps[b][:, :ns]).then_inc(sem_stage, 1)

        @block.scalar
        def _(sc):
            for wg in range(NW):
                b = wg % 2
                ns = windows[wg]["ns"]
                s0 = windows[wg]["slot0"]
                sc.wait_ge(sem_stage, wg + 1)
                sc.dma_start(out[:, s0:s0 + ns],
                             stage[b][:, :ns]).then_inc(sem_out, 16)

    nc.compile()
    return nc


# ---------------------------------------------------------------------- kernel
def kernel(u_f, v_f, u_w, v_w, src, dst):
    import ml_dtypes
    from concourse.bass_utils import run_bass_kernel_spmd

    src = np.asarray(src)
    dst = np.asarray(dst)
    u_f = np.asarray(u_f, np.float32)
    v_f = np.asarray(v_f, np.float32)

    deg_out = np.bincount(src, minlength=N).astype(np.float32)
    deg_in = np.bincount(dst, minlength=N).astype(np.float32)
    cout = np.maximum(deg_out, 1.0) ** -0.5
    cin = np.maximum(deg_in, 1.0) ** -0.5

    sched, per_core = _build_layout(src, dst, cout, cin)

    nc = _build_nc(sched)
    bf = ml_dtypes.bfloat16
    u16 = u_f.astype(bf)
    v16 = v_f.astype(bf)
    uw16 = np.asarray(u_w, np.float32).astype(bf)
    vw16 = np.asarray(v_w, np.float32).astype(bf)
    in_maps = []
    for k in range(N_CORES):
        in_maps.append({
            "u16": u16, "v16": v16, "uw": uw16, "vw": vw16,
            "idx": per_core[k]["idx"],
            "sval": per_core[k]["s"].astype(bf),
        })
    trace = bool(os.environ.get("KERNEL_TRACE"))
    res = run_bass_kernel_spmd(nc, in_maps, core_ids=list(range(N_CORES)),
                               trace=trace)
    if trace:
        print(f"HW exec time: {res.exec_time_ns} ns")
        kernel.last_profile = res.profile_json

    out_full = np.zeros((N, D), np.float32)
    for k in range(N_CORES):
        fm = res.results[k]["out"]            # [128, 2*SPC] feat-major
        rows = np.ascontiguousarray(fm.T)     # [2*SPC, 128]
        for phase in range(2):
            dsts = per_core[k]["dsts"][phase]
            out_full[dsts] = rows[phase * SPC:(phase + 1) * SPC]
    return out_full
